# revision 6
# baseline (speedup 1.0000x reference)
"""Additive (Bahdanau) attention on 8 Trainium2 NeuronCores.

Problem shapes (hardcoded): B=16, Q=64, K=512, DQ=DK=DV=512, H=256.

Strategy
--------
Data-parallel over B: 16 batches -> 8 cores x 2 batch "slots" per core.
The graph is specialized at build time on valid_lens: batches are sorted by
valid_len; slot A holds the 8 largest, slot B the 8 smallest.  Each slot's
key-extent EXT = max valid_len within the slot, so masked key tails are
simply never computed (sparse attention).

Per (batch, q) on device:
  qb_T[h,q] = (Wq^T queries_T)        (TensorE, contraction over d)
  kb_T[h,k] = (Wk^T keys_T)           (TensorE)
  t[h,k]    = kb_T[h,k] + qb_T[h,q]   (VectorE tensor_scalar, per-partition add)
  t         = tanh(t)                 (ScalarE, the roofline: 1 elem/cycle/lane)
  scores[q,:] += Wv_window^T t        (TensorE: stationary is a [128,64]
                                       zero-padded sliding window with Wv in
                                       column q, so row q of a PSUM [64,EXT]
                                       scores matrix accumulates directly)
  scores    += -30 * maskrow          (rank-1 matmul, masks k >= valid_len)
  E = exp(scores), S = rowsum         (ScalarE from PSUM, accum_out)
  attn_T    = E^T                     (TensorE transpose)
  out[q,:]  = (attn_T^T values) / S   (TensorE + VectorE drain with 1/S)

All device tensors are float16 (accumulation in fp32 PSUM); inputs are
transposed/cast/sliced on the host as part of sharding.
"""

import math
import os
import numpy as np

import concourse.bass as bass
import concourse.tile as tile
from concourse import mybir
from concourse.bass_utils import run_bass_kernel_spmd
from concourse.masks import make_identity

F16 = mybir.dt.float16
F32 = mybir.dt.float32

B, Q, K, D, H = 16, 64, 512, 512, 256
N_CORES = 8
MASK_ADD = -30.0  # exp(-30) ~ 1e-13: numerically zero vs unmasked weights
QG = 16  # q's per pipeline group


def _ceil_to(x, m):
    return ((x + m - 1) // m) * m


def _split_multi_waits(nc):
    """Workaround: this walrus build accepts only ONE sync wait per
    instruction.  Tile attaches one wait per cross-engine dependency, so
    hoist all but the last wait onto preceding same-engine
    InstEventSemaphore instructions (the same thing wait_ge lowers to)."""
    n = 0
    for fn in nc.m.functions:
        for blk in fn.blocks:
            out = []
            for ins in blk.instructions:
                si = getattr(ins, "sync_info", None)
                if si is not None and si.on_wait and len(si.on_wait) > 1:
                    waits = list(si.on_wait)
                    for w in waits[:-1]:
                        ev = mybir.InstEventSemaphore(
                            name=f"waitfix-{n}", ins=[], outs=[])
                        n += 1
                        ev.engine = ins.engine
                        ev.sync_info = mybir.SyncInfo(on_wait=[w], on_update=[])
                        out.append(ev)
                    si.on_wait = [waits[-1]]
                out.append(ins)
            blk.instructions = out
    return n


def build_nc(ea, eb):
    """Build the shared SPMD graph for slot extents (ea, eb); both even."""
    eac, ebc = _ceil_to(ea, 128), _ceil_to(eb, 128)
    nc = bass.Bass("TRN2")

    wq_d = nc.declare_dram_parameter("wq", [D, H], F16, isOutput=False)
    wk_d = nc.declare_dram_parameter("wk", [D, H], F16, isOutput=False)
    wv2_d = nc.declare_dram_parameter("wv2", [128, 254], F16, isOutput=False)
    qt_d = nc.declare_dram_parameter("qt", [D, 2 * Q], F16, isOutput=False)
    kta_d = nc.declare_dram_parameter("kta", [D, ea], F16, isOutput=False)
    ktb_d = nc.declare_dram_parameter("ktb", [D, eb], F16, isOutput=False)
    va_d = nc.declare_dram_parameter("va", [eac, 512], F16, isOutput=False)
    vb_d = nc.declare_dram_parameter("vb", [ebc, 512], F16, isOutput=False)
    ma_d = nc.declare_dram_parameter("maska", [1, ea], F16, isOutput=False)
    mb_d = nc.declare_dram_parameter("maskb", [1, eb], F16, isOutput=False)
    out_d = nc.declare_dram_parameter("out", [2, Q, 512], F32, isOutput=True)

    with tile.TileContext(nc) as tc, \
            tc.tile_pool(name="consts", bufs=1) as consts, \
            tc.tile_pool(name="ins", bufs=1) as ins, \
            tc.tile_pool(name="kb", bufs=1) as kbp, \
            tc.tile_pool(name="qb", bufs=1) as qbp, \
            tc.tile_pool(name="t0", bufs=3) as t0p, \
            tc.tile_pool(name="sm", bufs=1) as smp, \
            tc.tile_pool(name="outp", bufs=2) as outp, \
            tc.tile_pool(name="ps_kb", bufs=2, space="PSUM") as ps_kb, \
            tc.tile_pool(name="ps_qb", bufs=1, space="PSUM") as ps_qb, \
            tc.tile_pool(name="ps_sc", bufs=2, space="PSUM") as ps_sc, \
            tc.tile_pool(name="ps_et", bufs=1, space="PSUM") as ps_et, \
            tc.tile_pool(name="ps_o", bufs=1, space="PSUM") as ps_o:

        # Warm the ACT table set (tanh/exp share one set) during the DMA ramp.
        dummy = consts.tile([1, 2], F16, tag="dummy")
        nc.vector.memset(dummy, 0.0)
        nc.scalar.activation(dummy[:], dummy[:], mybir.ActivationFunctionType.Tanh)

        # --- constants / inputs to SBUF ---
        wq_sb = consts.tile([128, 4, H], F16, tag="wq")
        nc.sync.dma_start(out=wq_sb, in_=wq_d[:].rearrange("(t p) h -> p t h", p=128))
        wk_sb = consts.tile([128, 4, H], F16, tag="wk")
        nc.sync.dma_start(out=wk_sb, in_=wk_d[:].rearrange("(t p) h -> p t h", p=128))
        wv2_sb = consts.tile([128, 254], F16, tag="wv2")
        nc.sync.dma_start(out=wv2_sb, in_=wv2_d[:])
        qt_sb = consts.tile([128, 4, 2 * Q], F16, tag="qt")
        nc.sync.dma_start(out=qt_sb, in_=qt_d[:].rearrange("(t p) q -> p t q", p=128))
        ident = consts.tile([128, 128], F16, tag="ident")
        make_identity(nc, ident[:])
        ones = consts.tile([1, Q], F16, tag="ones")
        nc.vector.memset(ones, 1.0)

        slot_cfg = [
            (0, ea, eac, kta_d, va_d, ma_d),
            (1, eb, ebc, ktb_d, vb_d, mb_d),
        ]

        # Per-slot input tiles + projections (emitted first so DMAs/PE ramp early)
        kt_sb, v_sb, m_sb, kb_sb, qb_sb = {}, {}, {}, {}, {}
        for s, ext, extc, kt_d, v_d, m_d in slot_cfg:
            kt = ins.tile([128, 4, ext], F16, tag=f"kt{s}")
            nc.sync.dma_start(out=kt, in_=kt_d[:].rearrange("(t p) k -> p t k", p=128))
            vt = ins.tile([128, extc // 128, 512], F16, tag=f"v{s}")
            nc.sync.dma_start(out=vt, in_=v_d[:].rearrange("(t p) v -> p t v", p=128))
            mt = ins.tile([1, ext], F16, tag=f"m{s}")
            nc.sync.dma_start(out=mt, in_=m_d[:])
            kt_sb[s], v_sb[s], m_sb[s] = kt, vt, mt

            # projections: qb_T [128, 2, 64] and kb_T [128, 2, ext]
            qb_ps = ps_qb.tile([128, 2, Q], F32, tag="qb_ps")
            for ht in range(2):
                for dt in range(4):
                    nc.tensor.matmul(
                        qb_ps[:, ht, :],
                        wq_sb[:, dt, ht * 128:(ht + 1) * 128],
                        qt_sb[:, dt, s * Q:(s + 1) * Q],
                        start=(dt == 0), stop=(dt == 3),
                    )
            qb = qbp.tile([128, 2, Q], F32, tag=f"qb{s}")
            nc.vector.tensor_copy(qb[:], qb_ps[:])
            qb_sb[s] = qb

            kb = kbp.tile([128, 2, ext], F16, tag=f"kb{s}")
            for ht in range(2):
                kb_ps = ps_kb.tile([128, 512], F32, tag="kb_ps")
                for dt in range(4):
                    nc.tensor.matmul(
                        kb_ps[:, :ext],
                        wk_sb[:, dt, ht * 128:(ht + 1) * 128],
                        kt[:, dt, :],
                        start=(dt == 0), stop=(dt == 3),
                    )
                nc.vector.tensor_copy(kb[:, ht, :], kb_ps[:, :ext])
            kb_sb[s] = kb

        # Main loops + epilogues, slot A then slot B
        for s, ext, extc, kt_d, v_d, m_d in slot_cfg:
            kb, qb = kb_sb[s], qb_sb[s]
            scores = ps_sc.tile([Q, 512], F32, tag="scores")

            n_groups = Q // QG
            for g in range(n_groups):
                t0 = t0p.tile([128, QG, 2, ext], F16, tag="t0")
                for ql in range(QG):
                    q = g * QG + ql
                    for ht in range(2):
                        nc.vector.tensor_scalar_add(
                            out=t0[:, ql, ht, :],
                            in0=kb[:, ht, :],
                            scalar1=qb[:, ht, q:q + 1],
                        )
                nc.scalar.activation(
                    t0[:], t0[:], mybir.ActivationFunctionType.Tanh)
                for ql in range(QG):
                    q = g * QG + ql
                    for ht in range(2):
                        c0 = ht * 127 + 63 - q
                        nc.tensor.matmul(
                            scores[:, :ext],
                            wv2_sb[:, c0:c0 + Q],
                            t0[:, ql, ht, :],
                            start=(g == 0 and ql == 0 and ht == 0),
                            stop=False,
                        )
            # additive mask: scores[q, k] += -30 * (k >= valid_len)
            nc.tensor.matmul(scores[:, :ext], ones[:], m_sb[s][:],
                             start=False, stop=True)

            # softmax (no max-subtraction needed: |scores| < ~6)
            e_sb = smp.tile([Q, extc], F16, tag=f"e{s}")
            ssum = smp.tile([Q, 1], F32, tag=f"ssum{s}")
            sinv = smp.tile([Q, 1], F32, tag=f"sinv{s}")
            if extc > ext:
                nc.vector.memset(e_sb[:, ext:], 0.0)
            nc.scalar.activation(
                e_sb[:, :ext], scores[:, :ext],
                mybir.ActivationFunctionType.Exp, accum_out=ssum[:])
            nc.vector.reciprocal(sinv[:], ssum[:])

            # transpose E -> attn_T tiles, then attn_T^T @ values
            et = smp.tile([128, extc // 128, Q], F16, tag=f"et{s}")
            for kt_i in range(extc // 128):
                et_ps = ps_et.tile([128, Q], F16, tag="et_ps")
                nc.tensor.transpose(
                    et_ps[:], e_sb[:, kt_i * 128:(kt_i + 1) * 128], ident[:Q, :Q])
                nc.vector.tensor_copy(et[:, kt_i, :], et_ps[:])

            o_ps = ps_o.tile([Q, 512], F32, tag="o_ps")
            for kt_i in range(extc // 128):
                nc.tensor.matmul(
                    o_ps[:], et[:, kt_i, :], v_sb[s][:, kt_i, :],
                    start=(kt_i == 0), stop=(kt_i == extc // 128 - 1),
                )
            o_sb = outp.tile([Q, 512], F32, tag="o_sb")
            nc.vector.tensor_scalar_mul(out=o_sb[:], in0=o_ps[:], scalar1=sinv[:])
            nc.sync.dma_start(out=out_d[s], in_=o_sb[:])

    return nc


def _prep(inputs):
    """Shard + lay out inputs; returns (nc, in_maps, batch assignment)."""
    queries = np.asarray(inputs["queries"], np.float32)
    keys = np.asarray(inputs["keys"], np.float32)
    values = np.asarray(inputs["values"], np.float32)
    vlens = np.asarray(inputs["valid_lens"]).astype(np.int64)
    Wq = np.asarray(inputs["Wq"], np.float32)
    Wk = np.asarray(inputs["Wk"], np.float32)
    Wv = np.asarray(inputs["Wv"], np.float32)

    order = np.argsort(-vlens, kind="stable")
    slot_a, slot_b = order[:N_CORES], order[N_CORES:]
    ea = _ceil_to(int(vlens[slot_a].max()), 2)
    eb = _ceil_to(int(vlens[slot_b].max()), 2)
    eac, ebc = _ceil_to(ea, 128), _ceil_to(eb, 128)

    wq16 = Wq.astype(np.float16)
    wk16 = Wk.astype(np.float16)
    wv2 = np.zeros((128, 254), np.float16)
    wv2[:, 63] = Wv[:128].astype(np.float16)
    wv2[:, 127 + 63] = Wv[128:].astype(np.float16)

    in_maps = []
    for c in range(N_CORES):
        ba, bb = int(slot_a[c]), int(slot_b[c])
        la, lb = int(vlens[ba]), int(vlens[bb])
        qt = np.concatenate(
            [queries[ba].T, queries[bb].T], axis=1).astype(np.float16)
        kta = np.ascontiguousarray(keys[ba][:ea].T).astype(np.float16)
        ktb = np.ascontiguousarray(keys[bb][:eb].T).astype(np.float16)
        va = np.zeros((eac, 512), np.float16)
        va[:ea] = values[ba][:ea]
        vb = np.zeros((ebc, 512), np.float16)
        vb[:eb] = values[bb][:eb]
        maska = np.zeros((1, ea), np.float16)
        maska[0, la:] = MASK_ADD
        maskb = np.zeros((1, eb), np.float16)
        maskb[0, lb:] = MASK_ADD
        in_maps.append({
            "wq": wq16, "wk": wk16, "wv2": wv2, "qt": qt,
            "kta": kta, "ktb": ktb, "va": va, "vb": vb,
            "maska": maska, "maskb": maskb,
        })

    nc = build_nc(ea, eb)
    _split_multi_waits(nc)
    return nc, in_maps, slot_a, slot_b


def _run(inputs, trace=False):
    nc, in_maps, slot_a, slot_b = _prep(inputs)
    res = run_bass_kernel_spmd(
        nc, in_maps, core_ids=list(range(N_CORES)), trace=trace)
    out = np.empty((B, Q, 512), np.float32)
    for c in range(N_CORES):
        o = np.asarray(res.results[c]["out"], np.float32)
        out[int(slot_a[c])] = o[0]
        out[int(slot_b[c])] = o[1]
    return out, res


def kernel(**inputs):
    out, _ = _run(inputs, trace=False)
    return out


if __name__ == "__main__":
    rng = np.random.default_rng(0)
    demo = {
        "queries": rng.standard_normal((B, Q, D), dtype=np.float32),
        "keys": rng.standard_normal((B, K, D), dtype=np.float32),
        "values": rng.standard_normal((B, K, D), dtype=np.float32),
        "valid_lens": rng.integers(1, K + 1, size=(B,)).astype(np.int32),
        "Wq": rng.standard_normal((D, H), dtype=np.float32) / np.sqrt(D),
        "Wk": rng.standard_normal((D, H), dtype=np.float32) / np.sqrt(D),
        "Wv": rng.standard_normal((H,), dtype=np.float32) / np.sqrt(H),
    }
    print(kernel(**demo).shape)
